# revision 44
# baseline (speedup 1.0000x reference)
"""GAT (2-layer, 4-head) Trainium2 kernel, 8-way row-parallel, v2.

Transposed-matmul formulation: output rows i live on PSUM partitions, so
every matmul streams only the narrow value dimension (65-195 columns)
instead of the 512-wide i dimension -- PE work drops ~2.5x vs v1.

Per head, with s = f1_i + f2_j and masks A1 = adjT * [s>=0], A2 = adjT - A1:
  num_i/u2_i = r_i * M1 + (M_adj - M2')          r_i = exp(0.99 f1_i)
  M1  = sum_j A1[j,i] [w1 h|w1][j]   (masked matmul, lhsT = A1 chunk)
  M2' = sum_j A1[j,i] [w2 h|w2][j]   (same lhsT, packed in one 130-wide rhs)
  M_adj = sum_j adjT [w2 h|w2][j]    (unmasked, 3 heads packed 195-wide)
Head 0 skips the rank-1 factorization: t = exp(leaky_relu(s)) is computed
directly (Prelu+Exp on the otherwise-idle ScalarE), masked by adjT on DVE,
and used as lhsT against a plain [h|1] rhs -- no combines at all.
Masks cost 2 DVE ops (tensor_scalar is_ge + tensor_tensor mult, both 4x
mode) or one fused GPSIMD op; chunks are statically split across engines.
elu+1 is computed into fp16 (the -1 is folded into a rank-1 correction
matmul on W2aug), transposed via DMA-transpose, and layer 2 runs the same
transposed masked scheme after an fp16 AllGather of [512, 18] h2aug.
"""
import sys

for _p in ("/opt/trn_rl_repo", "/root/.axon_site/_ro/trn_rl_repo"):
    if _p not in sys.path:
        sys.path.insert(0, _p)

import numpy as np
import concourse.bass as bass
import concourse.bacc as bacc
import concourse.tile as tile
from concourse import mybir
from concourse.bass_utils import run_bass_kernel_spmd

F16 = mybir.dt.float16
F32 = mybir.dt.float32

N = 4096
NODE_DIM = 256
D = 64            # hidden per head
NH = 4            # heads
C2 = 16           # n_classes
NCORE = 8
R = N // NCORE    # rows per core (512)
P = 128
NCHUNK = N // P   # 32 j-chunks
NEG = 0.01        # leaky slope
DL1 = D + 1       # 65: [h*w | w]
NMH = 3           # masked heads (1..3)
W1REG = 2 * DL1   # 130: [w1h|w1|w2h|w2]
W1EXT = 3 * DL1   # 195: ... | -w2h|-w2 (adj-part folded into the group)
WADJ = NMH * DL1  # 195: adj-part, 3 heads stacked
DL2 = C2 + 1      # 17
W2REG = 2 * DL2   # 34
IS_GE = mybir.AluOpType.is_ge
MULT = mybir.AluOpType.mult
ADD = mybir.AluOpType.add
SUB = mybir.AluOpType.subtract
MIN = mybir.AluOpType.min
EXP = mybir.ActivationFunctionType.Exp
PRELU = mybir.ActivationFunctionType.Prelu
SIGN = mybir.ActivationFunctionType.Sign
RELU = mybir.ActivationFunctionType.Relu

# L1 masked-head chunks on the Pool engine (fused mask): (h, k) -> bool
POOL_L1 = {(h, k): ((k + 11 * h) % 4) == 0
           for h in (1, 2, 3) for k in range(NCHUNK)}
# head-0 chunks whose adjacency mult runs on Pool instead of DVE
TA_POOL = {k: k % 4 == 2 for k in range(NCHUNK)}
# L2 chunk paths: 0 = DVE 2-op, 1 = Pool fused, 2 = ACT sign+relu assist
L2_PATH = {k: (1 if (k * 7) % 32 < 5 else (2 if (k * 7) % 32 < 11 else 0))
           for k in range(NCHUNK)}


def build_kernel(use_collective=True):
    nc = bacc.Bacc("TRN2", target_bir_lowering=False, debug=False, num_devices=NCORE)

    adjt_d = nc.dram_tensor("adjt", [NCHUNK, P, R], F16, kind="ExternalInput")
    rhs1_d = nc.dram_tensor("rhs1", [NCHUNK, P, NMH, W1EXT], F16, kind="ExternalInput")
    rhsd_d = nc.dram_tensor("rhsd", [NCHUNK, P, DL1], F16, kind="ExternalInput")
    f1rep_d = nc.dram_tensor("f1rep", [P, NH, R], F16, kind="ExternalInput")
    negf2_d = nc.dram_tensor("negf2", [P, NMH, NCHUNK], F32, kind="ExternalInput")
    f2pos_d = nc.dram_tensor("f2pos", [P, NH, NCHUNK], F32, kind="ExternalInput")
    r1_d = nc.dram_tensor("r1", [P, 4, NMH], F32, kind="ExternalInput")
    w2aug_d = nc.dram_tensor("w2aug", [2, P, DL2 + 1], F16, kind="ExternalInput")
    w2corr_d = nc.dram_tensor("w2corr", [1, DL2 + 1], F16, kind="ExternalInput")
    out_d = nc.dram_tensor("out", [R, DL2], F32, kind="ExternalOutput")
    dbg_d = nc.dram_tensor("dbg", [R, DL2 + 1], F16, kind="ExternalOutput")

    with tile.TileContext(nc) as tc:
        with (
            tc.tile_pool(name="const", bufs=1) as const,
            tc.tile_pool(name="mask", bufs=12) as mask,
            tc.tile_pool(name="dir", bufs=12) as dirp,
            tc.tile_pool(name="comb", bufs=8) as comb,
            tc.tile_pool(name="small", bufs=8) as small,
            tc.tile_pool(name="psum", bufs=1, space="PSUM") as psum,
            tc.tile_pool(name="dram", bufs=1, space="DRAM") as dram,
        ):
            # ---------------- staged tensors ----------------
            adjt = const.tile([P, NCHUNK, R], F16)
            rhs1 = const.tile([P, NCHUNK, NMH, W1EXT], F16)
            rhsd = const.tile([P, NCHUNK, DL1], F16)
            f1rep = const.tile([P, NH, R], F16)
            negf2 = const.tile([P, NMH, NCHUNK], F32)
            f2pos = const.tile([P, NH, NCHUNK], F32)
            r1 = const.tile([P, 4, NMH], F32)
            w2aug = const.tile([P, 2, DL2 + 1], F16)
            w2corr = const.tile([1, DL2 + 1], F16)
            nc.sync.dma_start(out=f1rep, in_=f1rep_d[:, :, :])
            nc.sync.dma_start(out=negf2, in_=negf2_d[:, :, :])
            k0 = 0
            for PIECE in (2, 2, 4, 4, 4, 4, 4, 4, 4):
                nc.sync.dma_start(
                    out=adjt[:, k0:k0 + PIECE, :],
                    in_=adjt_d[k0:k0 + PIECE].rearrange("k p r -> p k r"))
                nc.sync.dma_start(
                    out=rhs1[:, k0:k0 + PIECE, :, :],
                    in_=rhs1_d[k0:k0 + PIECE].rearrange("k p v d -> p k v d"))
                nc.sync.dma_start(
                    out=rhsd[:, k0:k0 + PIECE, :],
                    in_=rhsd_d[k0:k0 + PIECE].rearrange("k p d -> p k d"))
                k0 += PIECE
                if k0 == 2:
                    nc.sync.dma_start(out=f2pos, in_=f2pos_d[:, :, :])
                elif k0 == 4:
                    nc.sync.dma_start(out=r1, in_=r1_d[:, :, :])
            for kk in range(2):
                nc.sync.dma_start(out=w2aug[:, kk, :], in_=w2aug_d[kk])
            nc.sync.dma_start(out=w2corr, in_=w2corr_d[:, :])
            ones = const.tile([1, R], F16)
            nc.vector.memset(ones, 1.0)
            zeros64 = const.tile([P, D], F32)
            nc.vector.memset(zeros64, 0.0)

            eluT = const.tile([P, 2, R], F16)      # layer-2 rhs, [d, kk, i]
            z2 = [const.tile([P, P], F16, name=f"z2_{i}")
                  for i in range(8)]  # (kk, it) pairs

            # L1 PSUM: per i-tile one [masked 130 | adj 195] tile + head-0 tile
            pall = [psum.tile([P, W1REG], F32, tag=f"pa{it}", name=f"pall{it}")
                    for it in range(4)]
            p0 = [psum.tile([P, DL1], F32, tag=f"p0{it}", name=f"p0_{it}") for it in range(4)]

            # head-0 chunks interleaved into the masked passes (spread ACT)
            h0_sched = {}
            for idx in range(NCHUNK):
                h0_sched.setdefault(idx % NMH, []).append(idx)
            h0_order = h0_sched[0] + h0_sched[1] + h0_sched[2]
            h0_first, h0_last = h0_order[0], h0_order[-1]

            def emit_head0_chunk(k):
                sp = dirp.tile([P, R], F16, tag="sp")
                t16 = dirp.tile([P, R], F16, tag="t16")
                ta = mask.tile([P, R], F16, tag="ta")
                nc.scalar.activation(out=sp, in_=f1rep[:, 0, :], func=PRELU,
                                     bias=f2pos[:, 0, k:k + 1], alpha=NEG)
                nc.scalar.activation(out=t16, in_=sp, func=EXP)
                if TA_POOL[k]:
                    nc.gpsimd.tensor_tensor(out=ta, in0=t16,
                                            in1=adjt[:, k, :], op=MULT)
                else:
                    nc.vector.tensor_tensor(ta, t16, adjt[:, k, :], MULT)
                for it in range(4):
                    nc.tensor.matmul(
                        out=p0[it][:, :], lhsT=ta[:, it * P:(it + 1) * P],
                        rhs=rhsd[:, k, :],
                        start=(k == h0_first), stop=(k == h0_last))

            def emit_elu(num_ap, den_ap, dst_ap, pool_ok=True):
                """dst = elu(num/den) + 1 in fp16 (num/den may be PSUM APs)."""
                rec = small.tile([P, 1], F32, tag="rec")
                nc.vector.reciprocal(out=rec, in_=den_ap)
                m0 = comb.tile([P, D], F32, tag="m0")
                e0 = comb.tile([P, D], F32, tag="e0")
                dd = comb.tile([P, D], F32, tag="dd")
                if pool_ok:  # mid-kernel: offload the subtract to Pool
                    nc.vector.tensor_scalar_min(m0, num_ap, 0.0)
                    nc.scalar.activation(out=e0, in_=m0, func=EXP, scale=rec)
                    nc.gpsimd.tensor_tensor(out=dd, in0=num_ap, in1=m0, op=SUB)
                else:  # drain: fewer cross-engine hops
                    nc.vector.tensor_scalar_min(m0, num_ap, 0.0)
                    nc.scalar.activation(out=e0, in_=m0, func=EXP, scale=rec)
                    nc.vector.tensor_tensor(dd, num_ap, m0, SUB)
                nc.vector.scalar_tensor_tensor(
                    out=dst_ap, in0=dd, scalar=rec, in1=e0, op0=MULT, op1=ADD)

            # ---------------- layer 1 ----------------
            # pass order (2, 3, 1): heads 2+3 (eluT kk=1) finish first so
            # their transposes + the kk=1 h2 matmul overlap head-1's pass
            for hpass, h in enumerate((2, 3, 1)):
                hm = h - 1
                for k in range(NCHUNK):
                    pm = mask.tile([P, R], F16, tag="pm")
                    a1 = mask.tile([P, R], F16, tag="a1")
                    nc.vector.tensor_scalar(
                        out=pm, in0=f1rep[:, h, :],
                        scalar1=negf2[:, hm, k:k + 1],
                        scalar2=None, op0=IS_GE)
                    if POOL_L1[(h, k)]:
                        nc.gpsimd.tensor_tensor(out=a1, in0=pm,
                                                in1=adjt[:, k, :], op=MULT)
                    else:
                        nc.vector.tensor_tensor(a1, pm, adjt[:, k, :], MULT)
                    for it in range(4):
                        # one accumulation group: masked 130-wide plus the
                        # negated adj-part into cols 65:130 (M2' - Madj)
                        nc.tensor.matmul(
                            out=pall[it][:, 0:W1REG],
                            lhsT=a1[:, it * P:(it + 1) * P],
                            rhs=rhs1[:, k, hm, 0:W1REG],
                            start=(k == 0), stop=False)
                        nc.tensor.matmul(
                            out=pall[it][:, DL1:W1REG],
                            lhsT=adjt[:, k, it * P:(it + 1) * P],
                            rhs=rhs1[:, k, hm, W1REG:W1EXT],
                            start=False, stop=(k == NCHUNK - 1))
                    if k in h0_sched[hpass]:
                        emit_head0_chunk(k)
                # combines + elu for head h
                for it in range(4):
                    dtmp = comb.tile([P, DL1], F32, tag="dt")
                    num = comb.tile([P, DL1], F32, tag="num")
                    nc.scalar.copy(dtmp, pall[it][:, DL1:W1REG])
                    nc.vector.scalar_tensor_tensor(
                        out=num, in0=pall[it][:, 0:DL1],
                        scalar=r1[:, it, hm:hm + 1],
                        in1=dtmp, op0=MULT, op1=SUB)
                    zi = 4 * (h // 2) + it        # z2 index: kk = h//2
                    emit_elu(num[:, 0:D], num[:, D:DL1],
                             z2[zi][:, (h % 2) * D:(h % 2) * D + D],
                             pool_ok=(h != 1))
                if h == 3:
                    # heads 2+3 done: kk=1 transposes overlap head-1's pass
                    for it in range(4):
                        q = nc.sync if it < 2 else nc.scalar
                        q.dma_start_transpose(
                            out=eluT[:, 1, it * P:(it + 1) * P], in_=z2[4 + it])

            # head-0 elu (kk=0, left half of z2[0..3])
            for it in range(4):
                emit_elu(p0[it][:, 0:D], p0[it][:, D:DL1], z2[it][:, 0:D],
                         pool_ok=False)
            for it in range(4):
                q = nc.sync if it < 2 else nc.scalar
                q.dma_start_transpose(
                    out=eluT[:, 0, it * P:(it + 1) * P], in_=z2[it])

            # ---------------- h2aug + AllGather ----------------
            h2t = psum.tile([DL2 + 1, R], F32, tag="p00")
            nc.tensor.matmul(out=h2t[:, :], lhsT=w2aug[:, 1, :],
                             rhs=eluT[:, 1, :], start=True, stop=False)
            nc.tensor.matmul(out=h2t[:, :], lhsT=w2aug[:, 0, :],
                             rhs=eluT[:, 0, :], start=False, stop=False)
            nc.tensor.matmul(out=h2t[:, :], lhsT=w2corr[:, :], rhs=ones,
                             start=False, stop=True)
            h2t16 = const.tile([32, R], F16)
            nc.vector.memset(h2t16, 0.0)
            nc.scalar.copy(h2t16[0:DL2 + 1, :], h2t)
            # f12rep for L2 masks (own rows, pre-gather); GPSIMD input must
            # start at partition 0, so bounce row 16 through a DMA first
            f12row = const.tile([1, R], F16)
            nc.sync.dma_start(out=f12row, in_=h2t16[C2:C2 + 1, :])
            f12rep = const.tile([P, R], F16)
            nc.gpsimd.partition_broadcast(out_ap=f12rep, in_ap=f12row)
            h2m = const.tile([P, 4, 32], F16)
            for it in range(4):
                q = nc.sync if it % 2 == 0 else nc.scalar
                q.dma_start_transpose(
                    out=h2m[:, it, :], in_=h2t16[:, it * P:(it + 1) * P])
            nc.scalar.dma_start(
                out=dbg_d[:, :].rearrange("(q p) d -> p q d", q=4),
                in_=h2m[:, :, 0:DL2 + 1])
            agout = dram.tile([N, DL2 + 1], F16)
            agin = dram.tile([R, DL2 + 1], F16)
            nc.scalar.dma_start(
                out=agin[:, :].rearrange("(q p) d -> p q d", q=4),
                in_=h2m[:, :, 0:DL2 + 1])
            if use_collective:
                nc.gpsimd.collective_compute(
                    "AllGather", mybir.AluOpType.bypass,
                    replica_groups=[list(range(NCORE))],
                    ins=[agin.opt()], outs=[agout.opt()])
            h2all = const.tile([P, NCHUNK, DL2 + 1], F16)
            # h2all chunk k=4c+g holds rows of core c, i-tile g
            if use_collective:
                agr = agout[:, :].rearrange("(k p) d -> p k d", p=P)
                for kq in range(4):
                    q = nc.sync if kq % 2 == 0 else nc.scalar
                    q.dma_start(out=h2all[:, kq * 8:(kq + 1) * 8, :],
                                in_=agr[:, kq * 8:(kq + 1) * 8, :])
            else:  # timing-only stand-in (TimelineSim is single-core):
                # per i-tile broadcast DMAs moving the same bytes the gather
                # would; chunks k=g (mod 4) land as each piece completes
                for g in range(4):
                    q = nc.sync if g % 2 == 0 else nc.scalar
                    q.dma_start(
                        out=h2all[:, g::4, :],
                        in_=agin[g * P:(g + 1) * P, :].unsqueeze(1)
                        .broadcast_to([P, NCORE, DL2 + 1]))

            # ---------------- layer 2 ----------------
            # rhs2[:, k, :]: [w1*h2 (16) | w1 | w2*h2 (16) | w2]  (34 wide)
            # prep + masks per q-group so work starts as gather pieces land
            w1c = small.tile([P, NCHUNK], F16, tag="w1c")
            w2c = small.tile([P, NCHUNK], F16, tag="w2c")
            nf22 = small.tile([P, NCHUNK], F32, tag="nf22")
            rhs2 = const.tile([P, NCHUNK, W2REG], F16)
            w1b = w1c[:, :].unsqueeze(2).broadcast_to([P, NCHUNK, C2])
            w2b = w2c[:, :].unsqueeze(2).broadcast_to([P, NCHUNK, C2])
            for g in range(4):
                sl = slice(g, NCHUNK, 4)
                nc.scalar.activation(out=w1c[:, sl], in_=h2all[:, sl, C2 + 1],
                                     func=EXP)
                nc.scalar.activation(out=w2c[:, sl], in_=h2all[:, sl, C2 + 1],
                                     func=EXP, scale=NEG)
                nc.vector.tensor_scalar_mul(nf22[:, sl],
                                            h2all[:, sl, C2 + 1], -1.0)
                nc.gpsimd.tensor_tensor(
                    out=rhs2[:, sl, 0:C2], in0=h2all[:, sl, 0:C2],
                    in1=w1b[:, sl, :], op=MULT)
                nc.vector.tensor_copy(rhs2[:, sl, C2], w1c[:, sl])
                nc.gpsimd.tensor_tensor(
                    out=rhs2[:, sl, DL2:DL2 + C2], in0=h2all[:, sl, 0:C2],
                    in1=w2b[:, sl, :], op=MULT)
                nc.vector.tensor_copy(rhs2[:, sl, DL2 + C2], w2c[:, sl])

            p2 = [psum.tile([P, W2REG], F32, tag=f"pa{it}", name=f"p2_{it}")
                  for it in range(4)]
            p2a = [psum.tile([P, DL2], F32, tag=f"p0{it}", name=f"p2a_{it}")
                   for it in range(4)]
            KORDER = [4 * j + g for g in range(4) for j in range(8)]
            for ki, k in enumerate(KORDER):
                path = L2_PATH[k]
                if path == 1:
                    pm = mask.tile([P, R], F16, tag="pm")
                    a1 = mask.tile([P, R], F16, tag="a1")
                    nc.vector.tensor_scalar(
                        out=pm, in0=f12rep, scalar1=nf22[:, k:k + 1],
                        scalar2=None, op0=IS_GE)
                    nc.gpsimd.tensor_tensor(out=a1, in0=pm,
                                            in1=adjt[:, k, :], op=MULT)
                elif path == 2:
                    qq = mask.tile([P, R], F16, tag="pm")
                    pm = mask.tile([P, R], F16, tag="pm")
                    a1 = mask.tile([P, R], F16, tag="a1")
                    nc.scalar.activation(out=qq, in_=f12rep, func=SIGN,
                                         bias=h2all[:, k, C2 + 1:C2 + 2])
                    nc.scalar.activation(out=pm, in_=qq, func=RELU)
                    nc.vector.tensor_tensor(a1, pm, adjt[:, k, :], MULT)
                else:
                    pm = mask.tile([P, R], F16, tag="pm")
                    a1 = mask.tile([P, R], F16, tag="a1")
                    nc.vector.tensor_scalar(
                        out=pm, in0=f12rep, scalar1=nf22[:, k:k + 1],
                        scalar2=None, op0=IS_GE)
                    nc.vector.tensor_tensor(a1, pm, adjt[:, k, :], MULT)
                for it in range(4):
                    nc.tensor.matmul(
                        out=p2[it][:, 0:W2REG],
                        lhsT=a1[:, it * P:(it + 1) * P], rhs=rhs2[:, k, :],
                        start=(ki == 0), stop=(ki == NCHUNK - 1))
                    nc.tensor.matmul(
                        out=p2a[it][:, :],
                        lhsT=adjt[:, k, it * P:(it + 1) * P],
                        rhs=rhs2[:, k, DL2:W2REG],
                        start=(ki == 0), stop=(ki == NCHUNK - 1))

            for it in range(4):
                r2 = small.tile([P, 1], F32, tag="r2")
                nc.scalar.activation(out=r2, in_=h2m[:, it, C2:C2 + 1],
                                     func=EXP, scale=1.0 - NEG)
                d2 = comb.tile([P, DL2], F32, tag="d2")
                o2 = comb.tile([P, DL2], F32, tag="o2")
                ma2 = comb.tile([P, DL2], F32, tag="ma2")
                nc.scalar.copy(ma2, p2a[it][:, :])
                nc.vector.scalar_tensor_tensor(
                    out=d2, in0=p2[it][:, DL2:W2REG], scalar=-1.0,
                    in1=ma2, op0=MULT, op1=ADD)
                nc.vector.scalar_tensor_tensor(
                    out=o2, in0=p2[it][:, 0:DL2], scalar=r2,
                    in1=d2, op0=MULT, op1=ADD)
                nc.sync.dma_start(out=out_d[it * P:(it + 1) * P, :], in_=o2)

    nc.compile()
    return nc


def host_prepare(x, adj_mat, W1, a1_1, a2_1, W2, a1_2, a2_2):
    """Build the per-core input maps (all fp32 math in numpy, fp16 staging)."""
    x = np.asarray(x, np.float32)
    adj = np.asarray(adj_mat)
    W1 = np.asarray(W1, np.float32)
    a1_1 = np.asarray(a1_1, np.float32)
    a2_1 = np.asarray(a2_1, np.float32)
    W2 = np.asarray(W2, np.float32)
    a1_2 = np.asarray(a1_2, np.float32)
    a2_2 = np.asarray(a2_2, np.float32)

    h = [x @ W1[k].T for k in range(NH)]              # [N, 64]
    f1 = [h[k] @ a1_1[k] for k in range(NH)]          # [N]
    f2 = [h[k] @ a2_1[k] for k in range(NH)]          # [N]

    # rhs1 [NCHUNK, P, NMH, W1REG] fp16 (heads 1..3)
    rhs1 = np.empty((N, NMH, W1EXT), np.float32)
    for hm, k in enumerate((1, 2, 3)):
        w1 = np.exp(f2[k])
        w2 = np.exp(NEG * f2[k])
        rhs1[:, hm, 0:D] = h[k] * w1[:, None]
        rhs1[:, hm, D] = w1
        rhs1[:, hm, DL1:DL1 + D] = h[k] * w2[:, None]
        rhs1[:, hm, DL1 + D] = w2
        rhs1[:, hm, W1REG:W1EXT] = -rhs1[:, hm, DL1:W1REG]
    rhs1 = rhs1.reshape(NCHUNK, P, NMH, W1EXT).astype(np.float16)

    # rhsd [NCHUNK, P, DL1] fp16 (head 0: [h | 1])
    rhsd = np.empty((N, DL1), np.float32)
    rhsd[:, 0:D] = h[0]
    rhsd[:, D] = 1.0
    rhsd = rhsd.reshape(NCHUNK, P, DL1).astype(np.float16)

    # negf2 [P, NMH, NCHUNK] f32, f2pos [P, NCHUNK] f32 (head 0)
    negf2 = np.empty((P, NMH, NCHUNK), np.float32)
    for hm, k in enumerate((1, 2, 3)):
        negf2[:, hm, :] = -f2[k].reshape(NCHUNK, P).T
    f2pos = np.empty((P, NH, NCHUNK), np.float32)
    for k in range(NH):
        f2pos[:, k, :] = f2[k].reshape(NCHUNK, P).T

    # W2aug [2, P, 18] fp16 + correction row
    w2aug = np.concatenate(
        [W2.T, (W2.T @ a1_2)[:, None], (W2.T @ a2_2)[:, None]], 1)  # [256, 18]
    w2corr = (-w2aug.sum(0, keepdims=True)).astype(np.float16)      # [1, 18]
    w2aug = w2aug.reshape(2, P, DL2 + 1).astype(np.float16)

    adj16 = adj.astype(np.float16)
    in_maps = []
    for c in range(NCORE):
        rows = slice(c * R, (c + 1) * R)
        adjt = np.ascontiguousarray(adj16[rows, :].T).reshape(NCHUNK, P, R)
        f1rep = np.empty((P, NH, R), np.float16)
        r1 = np.empty((P, 4, NMH), np.float32)
        for k in range(NH):
            f1rep[:, k, :] = f1[k][rows].astype(np.float16)[None, :]
        for hm, k in enumerate((1, 2, 3)):
            r1[:, :, hm] = np.exp((1.0 - NEG) * f1[k][rows]).reshape(4, P).T
        in_maps.append({
            "adjt": adjt, "rhs1": rhs1, "rhsd": rhsd, "f1rep": f1rep,
            "negf2": negf2, "f2pos": f2pos, "r1": r1, "w2aug": w2aug,
            "w2corr": w2corr,
        })
    return in_maps


_CACHE = {}


def kernel(trace=False, **inputs):
    in_maps = host_prepare(**inputs)
    if "nc" not in _CACHE:
        _CACHE["nc"] = build_kernel()
    res = run_bass_kernel_spmd(
        _CACHE["nc"], in_maps, core_ids=list(range(NCORE)), trace=trace)
    outs = []
    for c in range(NCORE):
        o = res.results[c]["out"]                     # [R, 17] f32
        outs.append(o[:, :C2] / o[:, C2:C2 + 1])
    full = np.concatenate(outs, 0).astype(np.float32)
    if trace:
        return full, res
    return full
